# revision 2
# baseline (speedup 1.0000x reference)
"""Trainium2 Bass kernel for nn_CausalPropagationAdjacency.

Problem (hardcoded shapes): B=4, T=12, N=512, D=128, L=4, H=64.
  - lag encoders (Linear D->H, ReLU, Linear H->D) over 4 lag slices, mean-agg
  - pairwise causal scorer: sigmoid(relu(src_i + tgt_j + bs1) @ Ws2 + bs2)
  - threshold 0.1, zero diagonal, enhanced = A + 0.5 A^2 + 0.25 A^3,
    normalize by global max per batch.

Sharding: 8 cores = 4 batch-pairs. Core c handles batch b=c//2, and scores
source rows [half*256, half*256+256) with half=c%2. The (256,512) adjacency
slab is AllGather'd within the pair; each core then computes the full
(512,512) enhanced matrix (hops are cheap) so no second collective is needed
for the global max. Host takes core 2b's full output.

SPMD note: all cores run one program; per-core behavior differs only through
input data (xlag = batch lag slices, xsrc = this core's half of them).

Pairwise stage: tgtT/srcT are kept in (D-partition, node-free) layout; per
source i one fused DVE tensor_scalar (add + max(,0), bf16 out) or ACT
Relu-with-bias produces relu(src_i + tgt + bs1) as a (128,512) bf16 tile; a
matmul against a sliding-window weight matrix (w2 embedded in column i%128 of
a zero 128x255 buffer) accumulates row i%128 of the (128,512) score block in
PSUM, turning the D-reduction into full-rate PE streaming.
"""

import os
import sys
import types
import numpy as np
import ml_dtypes

import concourse.bacc as bacc
import concourse.bass as bass
import concourse.bass_isa as bass_isa
import concourse.mybir as mybir
import concourse.tile as tile
from concourse.bass_utils import run_bass_kernel_spmd

B, T, N, D = 4, 12, 512, 128
L, H = 4, 64
THRESH = 0.1
NCORES = 8
NHALF = N // 2          # 256 source rows per core
NT = N // 128           # 4 node tiles
F32 = mybir.dt.float32
BF16 = mybir.dt.bfloat16
AF = mybir.ActivationFunctionType
ALU = mybir.AluOpType

# every ACT_EVERY-th pairwise tile goes to the scalar engine instead of DVE
ACT_EVERY = 5


def _build_nc():
    nc = bacc.Bacc("TRN2", target_bir_lowering=False, debug=False,
                   num_devices=NCORES)

    # ---- I/O ----
    xlag = nc.dram_tensor("xlag", [L, N, D], F32, kind="ExternalInput")
    xsrc = nc.dram_tensor("xsrc", [L, NHALF, D], F32, kind="ExternalInput")
    w1 = nc.dram_tensor("w1", [L, D, H], F32, kind="ExternalInput")
    w2 = nc.dram_tensor("w2", [L, H, D], F32, kind="ExternalInput")
    b1t = nc.dram_tensor("b1t", [H, L], F32, kind="ExternalInput")
    bmean = nc.dram_tensor("bmean", [D, 1], F32, kind="ExternalInput")
    ws1s = nc.dram_tensor("ws1s", [D, D], F32, kind="ExternalInput")
    ws1t = nc.dram_tensor("ws1t", [D, D], F32, kind="ExternalInput")
    bs1t = nc.dram_tensor("bs1t", [D, 1], F32, kind="ExternalInput")
    bs2t = nc.dram_tensor("bs2t", [128, 1], F32, kind="ExternalInput")
    zwin = nc.dram_tensor("zwin", [128, 255], BF16, kind="ExternalInput")
    identf = nc.dram_tensor("identf", [128, 128], F32, kind="ExternalInput")
    identh = nc.dram_tensor("identh", [128, 128], F32, kind="ExternalInput")
    outfull = nc.dram_tensor("outfull", [N, N], F32, kind="ExternalOutput")

    with tile.TileContext(nc) as tc:
        _emit(nc, tc, xlag, xsrc, w1, w2, b1t, bmean, ws1s, ws1t, bs1t, bs2t,
              zwin, identf, identh, outfull)
    nc.compile()
    return nc


def _emit(nc, tc, xlag, xsrc, w1, w2, b1t, bmean, ws1s, ws1t, bs1t, bs2t,
          zwin, identf, identh, outfull):
    from contextlib import ExitStack
    ctx = ExitStack()
    with ctx:
        consts = ctx.enter_context(tc.tile_pool(name="consts", bufs=1))
        xpool = ctx.enter_context(tc.tile_pool(name="xin", bufs=6))
        sb = ctx.enter_context(tc.tile_pool(name="sb", bufs=1))
        relup = ctx.enter_context(tc.tile_pool(name="relu", bufs=8))
        workp = ctx.enter_context(tc.tile_pool(name="work", bufs=4))
        psA = ctx.enter_context(tc.tile_pool(name="psA", bufs=2, space="PSUM"))
        psB = ctx.enter_context(tc.tile_pool(name="psB", bufs=2, space="PSUM"))
        psE = ctx.enter_context(tc.tile_pool(name="psE", bufs=4, space="PSUM"))
        dram = ctx.enter_context(tc.tile_pool(name="dram", bufs=1, space="DRAM"))

        # ---- constants / weights to SBUF ----
        idf = consts.tile([128, 128], F32, tag="idf")
        nc.sync.dma_start(idf[:], identf[:])
        idh = consts.tile([128, 128], F32, tag="idh")
        nc.sync.dma_start(idh[:], identh[:])
        zw = consts.tile([128, 255], BF16, tag="zw")
        nc.sync.dma_start(zw[:], zwin[:])
        w1sb = consts.tile([D, L, H], F32, tag="w1")
        nc.sync.dma_start(w1sb[:], w1.ap().rearrange("l d h -> d l h"))
        w2sb = consts.tile([H, L, D], F32, tag="w2")
        nc.sync.dma_start(w2sb[:], w2.ap().rearrange("l h d -> h l d"))
        ws1s_sb = consts.tile([D, D], F32, tag="ws1s")
        nc.sync.dma_start(ws1s_sb[:], ws1s[:])
        ws1t_sb = consts.tile([D, D], F32, tag="ws1t")
        nc.sync.dma_start(ws1t_sb[:], ws1t[:])
        b1sb = consts.tile([H, L], F32, tag="b1")
        nc.sync.dma_start(b1sb[:], b1t[:])
        bmean_sb = consts.tile([D, 1], F32, tag="bmean")
        nc.sync.dma_start(bmean_sb[:], bmean[:])
        bs1_sb = consts.tile([D, 1], F32, tag="bs1")
        nc.sync.dma_start(bs1_sb[:], bs1t[:])
        bs2_sb = consts.tile([128, 1], F32, tag="bs2")
        nc.sync.dma_start(bs2_sb[:], bs2t[:])

        # w1sb layout: [d, l, h] so w1sb[:, l, :] is the (128,64) lhsT
        # w2sb layout: [h, l, d] so w2sb[:, l, :] is the (64,128) lhsT

        # ---- encoder over full node set (target path) ----
        def encoder(src_dram, n_nodes, tag):
            """Returns aggT (D-part, n_nodes-free) fp32 SBUF tile."""
            ntiles = n_nodes // 128
            xT = {}
            for l in range(L):
                xTl = sb.tile([D, n_nodes], F32, tag=f"xT{tag}{l}")
                for nt in range(ntiles):
                    xin = xpool.tile([128, D], F32, tag="xin")
                    nc.sync.dma_start(
                        xin[:], src_dram[l, nt * 128:(nt + 1) * 128, :])
                    tp = psA.tile([128, 128], F32, tag="t")
                    nc.tensor.transpose(tp[:], xin[:], idf[:])
                    nc.scalar.copy(xTl[:, nt * 128:(nt + 1) * 128], tp[:])
                xT[l] = xTl
            encT = psB.tile([D, n_nodes], F32, tag="acc")
            for l in range(L):
                hT = psA.tile([H, n_nodes], F32, tag="t")
                nc.tensor.matmul(hT[:], w1sb[:, l, :], xT[l][:],
                                 start=True, stop=True)
                hsb = workp.tile([H, n_nodes], F32, tag=f"h{tag}")
                nc.scalar.activation(hsb[:], hT[:], AF.Relu,
                                     bias=b1sb[:, l:l + 1], scale=1.0)
                nc.tensor.matmul(encT[:], w2sb[:, l, :], hsb[:],
                                 start=(l == 0), stop=(l == L - 1))
            aggT = sb.tile([D, n_nodes], F32, tag=f"agg{tag}")
            nc.scalar.activation(aggT[:], encT[:], AF.Identity,
                                 bias=bmean_sb[:, 0:1], scale=1.0 / L)
            return aggT

        aggT_full = encoder(xlag, N, "f")
        aggT_src = encoder(xsrc, NHALF, "s")

        # ---- projections ----
        tgt_ps = psA.tile([D, N], F32, tag="t")
        nc.tensor.matmul(tgt_ps[:], ws1t_sb[:], aggT_full[:],
                         start=True, stop=True)
        tgtT_bf = sb.tile([D, N], BF16, tag="tgtbf")
        nc.vector.tensor_copy(tgtT_bf[:], tgt_ps[:])

        src_ps = psA.tile([D, NHALF], F32, tag="t")
        nc.tensor.matmul(src_ps[:], ws1s_sb[:], aggT_src[:],
                         start=True, stop=True)
        srcT = sb.tile([D, NHALF], F32, tag="srcf")
        nc.scalar.activation(srcT[:], src_ps[:], AF.Identity,
                             bias=bs1_sb[:, 0:1], scale=1.0)

        # ---- pairwise scoring: 256 source rows ----
        adj_bounce = dram.tile([NHALF, N], F32, tag="bnc")
        for blk in range(NHALF // 128):
            score_ps = psB.tile([128, N], F32, tag="acc")
            for p in range(128):
                i = blk * 128 + p
                rt = relup.tile([D, N], BF16, tag="rt")
                if i % ACT_EVERY == 2:
                    nc.scalar.activation(rt[:], tgtT_bf[:], AF.Relu,
                                         bias=srcT[:, i:i + 1], scale=1.0)
                else:
                    nc.vector.tensor_scalar(rt[:], tgtT_bf[:],
                                            srcT[:, i:i + 1], 0.0,
                                            ALU.add, ALU.max)
                nc.tensor.matmul(score_ps[:], zw[:, 127 - p:255 - p], rt[:],
                                 start=(p == 0), stop=(p == 127))
            score_sb = workp.tile([128, N], F32, tag="score")
            nc.scalar.activation(score_sb[:], score_ps[:], AF.Sigmoid,
                                 bias=bs2_sb[:, 0:1], scale=1.0)
            mask = workp.tile([128, N], F32, tag="mask")
            nc.vector.tensor_scalar(mask[:], score_sb[:], THRESH, None,
                                    ALU.is_gt)
            adjs = workp.tile([128, N], F32, tag="adjs")
            nc.vector.tensor_mul(adjs[:], score_sb[:], mask[:])
            nc.sync.dma_start(adj_bounce[blk * 128:(blk + 1) * 128, :],
                              adjs[:])

        # ---- all-gather adjacency slabs within the batch pair ----
        adj_full = dram.tile([N, N], F32, tag="full")
        nc.gpsimd.collective_compute(
            "AllGather", ALU.bypass,
            replica_groups=[[0, 1], [2, 3], [4, 5], [6, 7]],
            ins=[adj_bounce.opt()],
            outs=[adj_full.opt()],
        )

        # ---- load full adjacency, zero diagonal ----
        A = []
        for kt in range(NT):
            Ak = sb.tile([128, N], F32, tag=f"A{kt}")
            nc.sync.dma_start(Ak[:], adj_full[kt * 128:(kt + 1) * 128, :])
            # zero diag: keep where (j - p - 128*kt) != 0
            nc.gpsimd.affine_select(Ak[:], Ak[:], pattern=[[1, N]],
                                    compare_op=ALU.not_equal, fill=0.0,
                                    base=-(128 * kt), channel_multiplier=-1)
            A.append(Ak)

        # ---- A^T via PE transposes ----
        AT = []
        for kt in range(NT):
            ATk = sb.tile([128, N], F32, tag=f"AT{kt}")
            AT.append(ATk)
        for it in range(NT):
            for kt in range(NT):
                tp = psA.tile([128, 128], F32, tag="t")
                nc.tensor.transpose(
                    tp[:], A[it][:, kt * 128:(kt + 1) * 128], idf[:])
                if (it + kt) % 2 == 0:
                    nc.scalar.copy(AT[kt][:, it * 128:(it + 1) * 128], tp[:])
                else:
                    nc.vector.tensor_copy(
                        AT[kt][:, it * 128:(it + 1) * 128], tp[:])

        # ---- hops: a2 = A @ A ----
        a2 = []
        for it in range(NT):
            a2_ps = psA.tile([128, N], F32, tag="t")
            for kt in range(NT):
                nc.tensor.matmul(a2_ps[:],
                                 AT[kt][:, it * 128:(it + 1) * 128],
                                 A[kt][:], start=(kt == 0), stop=(kt == 3))
            a2sb = sb.tile([128, N], F32, tag=f"a2{it}")
            nc.vector.tensor_copy(a2sb[:], a2_ps[:])
            a2.append(a2sb)

        # ---- E = A @ a2 + 0.5 a2 + A  (accumulated in PSUM) ----
        E = []
        for it in range(NT):
            e_ps = psE.tile([128, N], F32, tag="E")
            for kt in range(NT):
                nc.tensor.matmul(e_ps[:],
                                 AT[kt][:, it * 128:(it + 1) * 128],
                                 a2[kt][:], start=(kt == 0), stop=False)
            nc.tensor.matmul(e_ps[:], idh[:], a2[it][:],
                             start=False, stop=False)
            nc.tensor.matmul(e_ps[:], idf[:], A[it][:],
                             start=False, stop=True)
            E.append(e_ps)

        # ---- global max + normalize ----
        mx4 = sb.tile([128, NT], F32, tag="mx4")
        for it in range(NT):
            nc.vector.reduce_max(mx4[:, it:it + 1], E[it][:],
                                 axis=mybir.AxisListType.X)
        mxp = sb.tile([128, 1], F32, tag="mxp")
        nc.vector.reduce_max(mxp[:], mx4[:], axis=mybir.AxisListType.X)
        mxall = sb.tile([128, 1], F32, tag="mxall")
        nc.gpsimd.partition_all_reduce(mxall[:], mxp[:], 128,
                                       bass_isa.ReduceOp.max)
        denom = sb.tile([128, 1], F32, tag="denom")
        nc.vector.tensor_scalar(denom[:], mxall[:], 1e-8, None, ALU.add)
        recip = sb.tile([128, 1], F32, tag="recip")
        nc.vector.reciprocal(recip[:], denom[:])

        for it in range(NT):
            ot = workp.tile([128, N], F32, tag="ot")
            nc.vector.tensor_scalar(ot[:], E[it][:], recip[:, 0:1], None,
                                    ALU.mult)
            nc.sync.dma_start(outfull[it * 128:(it + 1) * 128, :], ot[:])


_NC_CACHE = {}


def _get_nc():
    if "nc" not in _NC_CACHE:
        _NC_CACHE["nc"] = _build_nc()
    return _NC_CACHE["nc"]


def _install_ntff_hook():
    """Install the axon NTFF profiling hook if antenv.axon_hooks is missing."""
    try:
        from antenv.axon_hooks import get_axon_ntff_profile_hook  # noqa: F401
        return
    except ImportError:
        pass
    try:
        import importlib.util
        spec = importlib.util.spec_from_file_location(
            "trn_boot_mod", "/root/.axon_site/trn_agent_boot/trn_boot.py")
        tb = importlib.util.module_from_spec(spec)
        spec.loader.exec_module(tb)
        hook = tb._ntff_profile_via_ctypes("/opt/axon/libaxon_pjrt.so")
        m = types.ModuleType("antenv.axon_hooks")
        m.get_axon_ntff_profile_hook = lambda: hook
        m.set_axon_ntff_profile_hook = lambda h: None
        sys.modules["antenv.axon_hooks"] = m
    except Exception:
        pass


def _prep_in_maps(x, W1, b1, W2, b2, Ws1, bs1, Ws2, bs2):
    x = np.asarray(x, np.float32)
    W1 = np.asarray(W1, np.float32)
    b1 = np.asarray(b1, np.float32)
    W2 = np.asarray(W2, np.float32)
    b2 = np.asarray(b2, np.float32)
    Ws1 = np.asarray(Ws1, np.float32)
    bs1 = np.asarray(bs1, np.float32)
    Ws2 = np.asarray(Ws2, np.float32)
    bs2 = np.asarray(bs2, np.float32)

    Tdim = x.shape[1]
    lag_idx = [max(0, Tdim - 1 - l) for l in range(L)]
    xl = x[:, lag_idx]                         # (B, L, N, D)

    zwin = np.zeros((128, 255), np.float32)
    zwin[:, 127] = Ws2[:, 0]
    zwin = zwin.astype(ml_dtypes.bfloat16)

    common = {
        "w1": np.ascontiguousarray(W1),
        "w2": np.ascontiguousarray(W2),
        "b1t": np.ascontiguousarray(b1.T),                  # (H, L)
        "bmean": np.ascontiguousarray(b2.mean(axis=0)[:, None]),
        "ws1s": np.ascontiguousarray(Ws1[:D]),
        "ws1t": np.ascontiguousarray(Ws1[D:]),
        "bs1t": np.ascontiguousarray(bs1[:, None]),
        "bs2t": np.full((128, 1), bs2[0], np.float32),
        "zwin": zwin,
        "identf": np.eye(128, dtype=np.float32),
        "identh": (0.5 * np.eye(128)).astype(np.float32),
    }
    in_maps = []
    for c in range(NCORES):
        b, half = c // 2, c % 2
        m = dict(common)
        m["xlag"] = np.ascontiguousarray(xl[b])
        m["xsrc"] = np.ascontiguousarray(
            xl[b][:, half * NHALF:(half + 1) * NHALF, :])
        in_maps.append(m)
    return in_maps


def _run(inputs, trace=False):
    nc = _get_nc()
    in_maps = _prep_in_maps(**inputs)
    if trace:
        _install_ntff_hook()
    res = run_bass_kernel_spmd(nc, in_maps, core_ids=list(range(NCORES)),
                               trace=trace)
    out = np.stack([res.results[2 * b]["outfull"] for b in range(B)], axis=0)
    return out, res


def kernel(**inputs):
    out, _ = _run(inputs, trace=False)
    return out
